# revision 22
# baseline (speedup 1.0000x reference)
"""Multi-head causal attention (b=4, t=2048, k=1024, h=16) on 8 Trainium2 cores.

Sharding: core c = (batch b=c//2, head-group g=c%2). Each core computes one
batch x 8 heads; partial outputs (half heads each, and a further k01/k23
output-projection split) are summed on host.

Per-core kernel, v6. The tensor engine only sustains full clock under high
duty cycle and attention is a latency chain (ST -> exp on ACT -> PV), so:
  - q-major attention; consecutive k-tile pairs share one 2-bank psum tile
    so ONE exp covers both (20 exp calls/head instead of 40).
  - projection / output-projection matmuls interleave between attention
    groups as PE filler. Only the first Q/K chunk runs as prologue; V tiles
    and remaining Q/K chunks front-load into pair 0's groups in a
    dependency-safe order. Phase C splits into k01 (fills head 6) and k23
    (qc-gated, fills head 7) halves written to separate DRAM outputs that
    the host adds.
  - normalize: DVE reciprocal_approx_fast (denominator staged via SBUF),
    gpsimd partition_broadcast, DVE multiply; odd heads shift to
    partitions 64-127 via sbuf->sbuf DMA.
  PSUM: proj(2) + st(2x2-bank=4) + otp(2) = 8 banks.
"""
import sys

sys.path.insert(0, "/opt/trn_rl_repo")

import numpy as np
import ml_dtypes

import concourse.bass as bass
import concourse.mybir as mybir
import concourse.tile as tile
from concourse import bacc
from concourse.bass_utils import run_bass_kernel_spmd
from concourse.masks import make_upper_triangular

F32 = mybir.dt.float32
BF16 = mybir.dt.bfloat16
F8 = mybir.dt.float8e4
DR = mybir.MatmulPerfMode.DoubleRow
EXP = mybir.ActivationFunctionType.Exp
WSCALE = 16.0  # host pre-scales Wq/Wk/Wv by this to keep fp8 values normal

B, T, KD, NH, HS = 4, 2048, 1024, 16, 64
NCORES = 8


def build_nc(t=T, dl=512, hl=8, kd=KD):
    nk = kd // 128       # contraction tiles for projections
    mt = t // 128        # t tiles (k-position tiles in attention)
    dt = dl // 128       # local-dim tiles (head pairs)
    nqc = t // 512       # q chunks
    scale = 1.0 / float(np.sqrt(kd)) / (WSCALE * WSCALE)

    nc = bacc.Bacc("TRN2", target_bir_lowering=False, debug=False, num_devices=NCORES)
    xt_d = nc.dram_tensor("xt", [kd, t], F8, kind="ExternalInput")
    xtb_d = nc.dram_tensor("xtb", [kd, t], BF16, kind="ExternalInput")
    wq_d = nc.dram_tensor("wq", [kd, dl], F8, kind="ExternalInput")
    wk_d = nc.dram_tensor("wk", [kd, dl], F8, kind="ExternalInput")
    wv_d = nc.dram_tensor("wv", [kd, dl], BF16, kind="ExternalInput")
    wo_d = nc.dram_tensor("wo", [dl, kd], BF16, kind="ExternalInput")
    out_d = nc.dram_tensor("out", [t, kd], F32, kind="ExternalOutput")
    out2_d = nc.dram_tensor("out2", [t, kd], F32, kind="ExternalOutput")

    with tile.TileContext(nc) as tc:
        with (
            tc.tile_pool(name="persist", bufs=1) as pp,
            tc.tile_pool(name="misc", bufs=1) as mp,
            tc.tile_pool(name="pbe", bufs=8) as pbe,
            tc.tile_pool(name="pbm", bufs=4) as pbm,
            tc.tile_pool(name="pco", bufs=3) as pco,
            tc.tile_pool(name="psum", bufs=1, space="PSUM") as psp,
        ):
            qt_s = pp.tile([128, dt, t], BF16)
            kt_s = pp.tile([128, dt, t], BF16)
            v_s = pp.tile([128, mt, hl, 65], BF16)
            ot_s = pp.tile([128, dt, t], BF16)
            xt_s = pp.tile([128, nk, t], F8)
            xtb_s = pp.tile([128, nk, t], BF16)
            wq_s = pp.tile([128, nk, dl], F8)
            wk_s = pp.tile([128, nk, dl], F8)
            wv_s = pp.tile([128, nk, dl], BF16)
            wo_s = pp.tile([128, dt, kd], BF16)
            mask_f = mp.tile([128, 128], F32)
            mask_t = mp.tile([128, 128], BF16)
            make_upper_triangular(nc, mask_f[:, :], val=1.0, diag=True)
            nc.vector.tensor_copy(mask_t[:, :], mask_f[:, :])
            nc.vector.memset(v_s[:, :, :, 64], 1.0)

            # --------------- input DMA (fine-grained for fast start) -------
            wq_r = wq_d[:, :].rearrange("(n p) d -> p n d", p=128)
            wk_r = wk_d[:, :].rearrange("(n p) d -> p n d", p=128)
            wv_r = wv_d[:, :].rearrange("(n p) d -> p n d", p=128)
            xt_r = xt_d[:, :].rearrange("(n p) t -> p n t", p=128)
            for k in range(nk):
                nc.scalar.dma_start(wq_s[:, k, :], wq_r[:, k, :])
            for k in range(nk):
                nc.sync.dma_start(xt_s[:, k, 0:512], xt_r[:, k, 0:512])
            for k in range(nk):
                nc.scalar.dma_start(wk_s[:, k, :], wk_r[:, k, :])
            for n in range(1, t // 512):
                nc.sync.dma_start(
                    xt_s[:, :, 512 * n : 512 * n + 512],
                    xt_r[:, :, 512 * n : 512 * n + 512],
                )
            nc.scalar.dma_start(wv_s[:, :, :], wv_r[:, :, :])
            # xtb rides the gpsimd software-DGE queue: third parallel stream
            xtb_r = xtb_d[:, :].rearrange("(n p) t -> p n t", p=128)
            for n in range(t // 512):
                nc.gpsimd.dma_start(
                    xtb_s[:, :, 512 * n : 512 * n + 512],
                    xtb_r[:, :, 512 * n : 512 * n + 512],
                )
            nc.scalar.dma_start(
                wo_s[:, :, :], wo_d[:, :].rearrange("(n p) o -> p n o", p=128)
            )

            # --------------- filler emitters -------------------------------
            cnt = [0]

            def emit_qk(w_s, o_s, pair, n, on_act=False):
                cols = slice(512 * n, 512 * n + 512)
                ps = psp.tile([128, 512], F32, name=f"pj{cnt[0]}", tag="proj", bufs=2)
                cnt[0] += 1
                for k in range(nk // 2):
                    nc.tensor.matmul(
                        ps[:, :],
                        w_s[:, 2 * k : 2 * k + 2, 128 * pair : 128 * pair + 128],
                        xt_s[:, 2 * k : 2 * k + 2, cols],
                        start=(k == 0),
                        stop=(k == nk // 2 - 1),
                        perf_mode=DR,
                    )
                if on_act:
                    nc.scalar.copy(o_s[:, pair, cols], ps[:, :])
                else:
                    nc.vector.tensor_copy(o_s[:, pair, cols], ps[:, :])

            def emit_v(m, on_act=False):
                ps = psp.tile([128, 512], F32, name=f"pv{cnt[0]}", tag="proj", bufs=2)
                cnt[0] += 1
                for k in range(nk):
                    nc.tensor.matmul(
                        ps[:, :],
                        xtb_s[:, k, 128 * m : 128 * m + 128],
                        wv_s[:, k, :],
                        start=(k == 0),
                        stop=(k == nk - 1),
                    )
                src = ps[:, :].rearrange("p (h d) -> p h d", h=hl)
                if on_act:
                    nc.scalar.copy(v_s[:, m, :, 0:64], src)
                else:
                    nc.vector.tensor_copy(v_s[:, m, :, 0:64], src)

            def emit_c(m, ks, dst_d):
                """Half of phase C for t-tile m, contracting head-pairs `ks`."""
                ob = pco.tile([128, kd], F32, name=f"ob{cnt[0]}", tag="ob")
                cnt[0] += 1
                for c in range(kd // 512):
                    ps = psp.tile(
                        [128, 512], F32, name=f"pc{cnt[0]}", tag="proj", bufs=2
                    )
                    cnt[0] += 1
                    for j, k in enumerate(ks):
                        nc.tensor.matmul(
                            ps[:, :],
                            ot_s[:, k, 128 * m : 128 * m + 128],
                            wo_s[:, k, 512 * c : 512 * c + 512],
                            start=(j == 0),
                            stop=(j == len(ks) - 1),
                        )
                    nc.vector.tensor_copy(ob[:, 512 * c : 512 * c + 512], ps[:, :])
                nc.sync.dma_start(dst_d[128 * m : 128 * m + 128, :], ob[:, :])

            # --------------- prologue: first Q/K chunk only ----------------
            emit_qk(wq_s, qt_s, 0, 0, on_act=True)
            emit_qk(wk_s, kt_s, 0, 0, on_act=True)

            # --------------- fused attention + filler ----------------------
            def emit_pv(h, ki, qc, a, b, ex, exo, otp):
                """PV for one (ki, qc) unit; ex columns [exo, exo + b - a)."""
                nc.tensor.matmul(
                    otp[0:65, a - 512 * qc : b - 512 * qc],
                    v_s[:, ki, h, :],
                    ex[:, exo : exo + b - a],
                    start=(ki == 0),
                    stop=(ki == 4 * qc + 3),
                )
                if ki != 4 * qc + 3:
                    return False
                mh, ph = h // 2, 64 * (h % 2)
                den = pbm.tile([1, 512], F32, name=f"dn{h}_{qc}", tag="den")
                nc.vector.tensor_copy(den[:, :], otp[64:65, :])
                rec = pbm.tile([1, 512], F32, name=f"rc{h}_{qc}", tag="rec")
                nc.vector.reciprocal_approx_fast(rec[:, :], den[:, :])
                bc = pbm.tile([64, 512], F32, name=f"bc{h}_{qc}", tag="bc")
                nc.gpsimd.partition_broadcast(bc[:, :], rec[0:1, :])
                cols = slice(512 * qc, 512 * qc + 512)
                if ph == 0:
                    nc.vector.tensor_mul(ot_s[0:64, mh, cols], otp[0:64, :], bc[:, :])
                else:
                    sc = pbm.tile([64, 512], BF16, name=f"sc{h}_{qc}", tag="sc")
                    nc.vector.tensor_mul(sc[:, :], otp[0:64, :], bc[:, :])
                    nc.sync.dma_start(ot_s[64:128, mh, cols], sc[:, :])
                return True

            ngrp_head = sum((4 * qc + 4) // 2 for qc in range(nqc))  # 20

            for p in range(dt):
                front = []
                spread = []
                if p == 0:
                    # dependency-safe front-load order (2 items per group):
                    # QK chunk n must precede attention q-chunk n; V tiles
                    # stream ahead of their PV consumers.
                    front = [
                        lambda: emit_v(0),
                        lambda: emit_v(1),
                        lambda: emit_qk(wq_s, qt_s, 0, 1),
                        lambda: emit_qk(wk_s, kt_s, 0, 1),
                        lambda: emit_v(2),
                        lambda: emit_v(3),
                        lambda: emit_v(4),
                        lambda: emit_qk(wq_s, qt_s, 0, 2),
                        lambda: emit_qk(wk_s, kt_s, 0, 2),
                        lambda: emit_v(5),
                        lambda: emit_v(6),
                        lambda: emit_v(7),
                        lambda: emit_qk(wq_s, qt_s, 0, 3),
                        lambda: emit_qk(wk_s, kt_s, 0, 3),
                    ] + [(lambda m=m: emit_v(m)) for m in range(8, mt)]
                if p < dt - 1:
                    for n in range(4):
                        spread.append(lambda n=n, p=p: emit_qk(wq_s, qt_s, p + 1, n))
                        spread.append(lambda n=n, p=p: emit_qk(wk_s, kt_s, p + 1, n))
                if p == dt - 1:
                    spread += [
                        (lambda m=m: emit_c(m, (0, 1), out2_d)) for m in range(mt)
                    ]
                fr = [0]
                fi = [0]
                pui = [0]
                npace = ngrp_head if p == dt - 1 else 2 * ngrp_head

                def maybe_fill():
                    pui[0] += 1
                    took = 0
                    while fr[0] < len(front) and took < 2:
                        front[fr[0]]()
                        fr[0] += 1
                        took += 1
                    if took:
                        return
                    want = pui[0] * len(spread) // npace
                    while fi[0] < min(want, len(spread)):
                        spread[fi[0]]()
                        fi[0] += 1

                for h in (2 * p, 2 * p + 1):
                    mh, ph = h // 2, 64 * (h % 2)
                    if p == dt - 1 and h == 2 * p + 1:
                        while fi[0] < len(spread):
                            spread[fi[0]]()
                            fi[0] += 1
                        spread = []
                        fi[0] = 0
                        pui[0] = 0
                    # deferred-PV depth: deep for the very first head so the
                    # ST/exp stream never blocks on the late-arriving V tiles
                    pv_depth = 12 if (p == 0 and h == 0) else 2
                    pv_pending = []

                    def drain_pv(limit):
                        while len(pv_pending) > limit:
                            args = pv_pending.pop(0)
                            done = emit_pv(*args)
                            if done and p == dt - 1 and h == 2 * p + 1:
                                dqc = args[2]
                                spread.extend(
                                    (lambda m=m: emit_c(m, (2, 3), out_d))
                                    for m in range(4 * dqc, 4 * dqc + 4)
                                )
                    for qc in range(nqc):
                        # otp bufs=2: before taking qc's slot (= qc-2's), all
                        # deferred PVs/normalize touching qc-2 must be emitted
                        while pv_pending and pv_pending[0][2] <= qc - 2:
                            drain_pv(len(pv_pending) - 1)
                        otp = psp.tile(
                            [65, 512], F32, name=f"otp{h}_{qc}", tag="ot", bufs=2
                        )
                        for ki0 in range(0, 4 * qc + 4, 2):
                            st = psp.tile(
                                [128, 1024], F32, name=f"st{h}_{ki0}_{qc}",
                                tag="st", bufs=2,
                            )
                            ex = pbe.tile(
                                [128, 1024], BF16, name=f"ex{h}_{ki0}_{qc}",
                                tag="ex",
                            )
                            # place the two units contiguously (no unwritten
                            # psum gap for exp): unit 2 at w1 if both fit in
                            # bank 0, else at the bank-1 boundary
                            b = 512 * qc + 512
                            a1 = max(128 * ki0, 512 * qc)
                            a2 = max(128 * (ki0 + 1), 512 * qc)
                            w1, w2 = b - a1, b - a2
                            o2 = w1 if w1 + w2 <= 512 else 512
                            ws = [(ki0, a1, 0), (ki0 + 1, a2, o2)]
                            for ki, a, off in ws:
                                nc.tensor.matmul(
                                    st[:, off : off + b - a],
                                    kt_s[ph : ph + 64, mh, 128 * ki : 128 * ki + 128],
                                    qt_s[ph : ph + 64, mh, a:b],
                                    start=True,
                                    stop=True,
                                )
                            nc.scalar.activation(
                                ex[:, 0 : o2 + w2], st[:, 0 : o2 + w2],
                                EXP, scale=scale,
                            )
                            for ki, a, off in ws:
                                if a == 128 * ki:
                                    nc.vector.tensor_mul(
                                        ex[:, off : off + 128],
                                        ex[:, off : off + 128],
                                        mask_t[:, :],
                                    )
                            maybe_fill()
                            pv_pending.extend(
                                (h, ki, qc, a, b, ex, off, otp)
                                for ki, a, off in ws
                            )
                            drain_pv(pv_depth)
                    drain_pv(0)
                while fi[0] < len(spread):
                    spread[fi[0]]()
                    fi[0] += 1

    nc.finalize()
    return nc


_NC_CACHE = {}


def _get_nc(key=(T, 512, 8, KD)):
    if key not in _NC_CACHE:
        _NC_CACHE[key] = build_nc(*key)
    return _NC_CACHE[key]


def make_in_maps(x, Wq, Wk, Wv, Wo, dl=512):
    in_maps = []
    for c in range(NCORES):
        b, g = c // 2, c % 2
        rows = slice(dl * g, dl * (g + 1))
        in_maps.append(
            {
                "xt": np.ascontiguousarray(x[b].T).astype(ml_dtypes.float8_e4m3fn),
                "xtb": np.ascontiguousarray(x[b].T).astype(ml_dtypes.bfloat16),
                "wq": np.ascontiguousarray(Wq[rows, :].T * WSCALE).astype(
                    ml_dtypes.float8_e4m3fn
                ),
                "wk": np.ascontiguousarray(Wk[rows, :].T * WSCALE).astype(
                    ml_dtypes.float8_e4m3fn
                ),
                "wv": np.ascontiguousarray(Wv[rows, :].T).astype(ml_dtypes.bfloat16),
                "wo": np.ascontiguousarray(Wo[:, rows].T).astype(ml_dtypes.bfloat16),
            }
        )
    return in_maps


def run_spmd(x, Wq, Wk, Wv, Wo, trace=False):
    nc = _get_nc()
    in_maps = make_in_maps(x, Wq, Wk, Wv, Wo)
    res = run_bass_kernel_spmd(nc, in_maps, list(range(NCORES)), trace=trace)
    outs = [
        res.results[c]["out"] + res.results[c]["out2"] for c in range(NCORES)
    ]
    final = np.stack([outs[2 * b] + outs[2 * b + 1] for b in range(B)])
    return final.astype(np.float32), res


def kernel(x, Wq, Wk, Wv, Wo):
    x = np.asarray(x, dtype=np.float32)
    Wq = np.asarray(Wq, dtype=np.float32)
    Wk = np.asarray(Wk, dtype=np.float32)
    Wv = np.asarray(Wv, dtype=np.float32)
    Wo = np.asarray(Wo, dtype=np.float32)
    out, _ = run_spmd(x, Wq, Wk, Wv, Wo)
    return out


# revision 23
# speedup vs baseline: 1.0260x; 1.0260x over previous
"""Multi-head causal attention (b=4, t=2048, k=1024, h=16) on 8 Trainium2 cores.

Sharding: core c = (batch b=c//2, head-group g=c%2). Each core computes one
batch x 8 heads; partial outputs (half heads each, and a further k01/k23
output-projection split) are summed on host.

Per-core kernel, v6. The tensor engine only sustains full clock under high
duty cycle and attention is a latency chain (ST -> exp on ACT -> PV), so:
  - q-major attention; consecutive k-tile pairs share one 2-bank psum tile
    so ONE exp covers both (20 exp calls/head instead of 40).
  - projection / output-projection matmuls interleave between attention
    groups as PE filler. Only the first Q/K chunk runs as prologue; V tiles
    and remaining Q/K chunks front-load into pair 0's groups in a
    dependency-safe order. Phase C splits into k01 (fills head 6) and k23
    (qc-gated, fills head 7) halves written to separate DRAM outputs that
    the host adds.
  - normalize: DVE reciprocal_approx_fast (denominator staged via SBUF),
    gpsimd partition_broadcast, DVE multiply; odd heads shift to
    partitions 64-127 via sbuf->sbuf DMA.
  PSUM: proj(2) + st(2x2-bank=4) + otp(2) = 8 banks.
"""
import sys

sys.path.insert(0, "/opt/trn_rl_repo")

import numpy as np
import ml_dtypes

import concourse.bass as bass
import concourse.mybir as mybir
import concourse.tile as tile
from concourse import bacc
from concourse.bass_utils import run_bass_kernel_spmd
from concourse.masks import make_upper_triangular

F32 = mybir.dt.float32
BF16 = mybir.dt.bfloat16
F8 = mybir.dt.float8e4
DR = mybir.MatmulPerfMode.DoubleRow
EXP = mybir.ActivationFunctionType.Exp
WSCALE = 16.0  # host pre-scales Wq/Wk/Wv by this to keep fp8 values normal

B, T, KD, NH, HS = 4, 2048, 1024, 16, 64
NCORES = 8


def build_nc(t=T, dl=512, hl=8, kd=KD):
    nk = kd // 128       # contraction tiles for projections
    mt = t // 128        # t tiles (k-position tiles in attention)
    dt = dl // 128       # local-dim tiles (head pairs)
    nqc = t // 512       # q chunks
    scale = 1.0 / float(np.sqrt(kd)) / (WSCALE * WSCALE)

    nc = bacc.Bacc("TRN2", target_bir_lowering=False, debug=False, num_devices=NCORES)
    xt_d = nc.dram_tensor("xt", [kd, t], F8, kind="ExternalInput")
    xtb_d = nc.dram_tensor("xtb", [kd, t], BF16, kind="ExternalInput")
    wq_d = nc.dram_tensor("wq", [kd, dl], F8, kind="ExternalInput")
    wk_d = nc.dram_tensor("wk", [kd, dl], F8, kind="ExternalInput")
    wv_d = nc.dram_tensor("wv", [kd, dl], BF16, kind="ExternalInput")
    wo_d = nc.dram_tensor("wo", [dl, kd], BF16, kind="ExternalInput")
    out_d = nc.dram_tensor("out", [t, kd], F32, kind="ExternalOutput")
    out2_d = nc.dram_tensor("out2", [t, kd], F32, kind="ExternalOutput")

    with tile.TileContext(nc) as tc:
        with (
            tc.tile_pool(name="persist", bufs=1) as pp,
            tc.tile_pool(name="misc", bufs=1) as mp,
            tc.tile_pool(name="pbe", bufs=8) as pbe,
            tc.tile_pool(name="pbm", bufs=4) as pbm,
            tc.tile_pool(name="pco", bufs=3) as pco,
            tc.tile_pool(name="psum", bufs=1, space="PSUM") as psp,
        ):
            qt_s = pp.tile([128, dt, t], BF16)
            kt_s = pp.tile([128, dt, t], BF16)
            v_s = pp.tile([128, mt, hl, 65], BF16)
            ot_s = pp.tile([128, dt, t], BF16)
            xt_s = pp.tile([128, nk, t], F8)
            xtb_s = pp.tile([128, nk, t], BF16)
            wq_s = pp.tile([128, nk, dl], F8)
            wk_s = pp.tile([128, nk, dl], F8)
            wv_s = pp.tile([128, nk, dl], BF16)
            wo_s = pp.tile([128, dt, kd], BF16)
            mask_f = mp.tile([128, 128], F32)
            mask_t = mp.tile([128, 128], BF16)
            make_upper_triangular(nc, mask_f[:, :], val=1.0, diag=True)
            nc.vector.tensor_copy(mask_t[:, :], mask_f[:, :])
            nc.vector.memset(v_s[:, :, :, 64], 1.0)

            # --------------- input DMA (fine-grained for fast start) -------
            wq_r = wq_d[:, :].rearrange("(n p) d -> p n d", p=128)
            wk_r = wk_d[:, :].rearrange("(n p) d -> p n d", p=128)
            wv_r = wv_d[:, :].rearrange("(n p) d -> p n d", p=128)
            xt_r = xt_d[:, :].rearrange("(n p) t -> p n t", p=128)
            for k in range(nk):
                nc.scalar.dma_start(wq_s[:, k, :], wq_r[:, k, :])
            for k in range(nk):
                nc.sync.dma_start(xt_s[:, k, 0:512], xt_r[:, k, 0:512])
            for k in range(nk):
                nc.scalar.dma_start(wk_s[:, k, :], wk_r[:, k, :])
            for n in range(1, t // 512):
                nc.sync.dma_start(
                    xt_s[:, :, 512 * n : 512 * n + 512],
                    xt_r[:, :, 512 * n : 512 * n + 512],
                )
            nc.scalar.dma_start(wv_s[:, :, :], wv_r[:, :, :])
            # xtb split across both hw queues to balance total bytes
            xtb_r = xtb_d[:, :].rearrange("(n p) t -> p n t", p=128)
            for n in range(t // 512):
                eng = nc.sync if n < 2 else nc.scalar
                eng.dma_start(
                    xtb_s[:, :, 512 * n : 512 * n + 512],
                    xtb_r[:, :, 512 * n : 512 * n + 512],
                )
            nc.scalar.dma_start(
                wo_s[:, :, :], wo_d[:, :].rearrange("(n p) o -> p n o", p=128)
            )

            # --------------- filler emitters -------------------------------
            cnt = [0]

            def emit_qk(w_s, o_s, pair, n, on_act=False):
                cols = slice(512 * n, 512 * n + 512)
                ps = psp.tile([128, 512], F32, name=f"pj{cnt[0]}", tag="proj", bufs=2)
                cnt[0] += 1
                for k in range(nk // 2):
                    nc.tensor.matmul(
                        ps[:, :],
                        w_s[:, 2 * k : 2 * k + 2, 128 * pair : 128 * pair + 128],
                        xt_s[:, 2 * k : 2 * k + 2, cols],
                        start=(k == 0),
                        stop=(k == nk // 2 - 1),
                        perf_mode=DR,
                    )
                if on_act:
                    nc.scalar.copy(o_s[:, pair, cols], ps[:, :])
                else:
                    nc.vector.tensor_copy(o_s[:, pair, cols], ps[:, :])

            def emit_v(m, on_act=False):
                ps = psp.tile([128, 512], F32, name=f"pv{cnt[0]}", tag="proj", bufs=2)
                cnt[0] += 1
                for k in range(nk):
                    nc.tensor.matmul(
                        ps[:, :],
                        xtb_s[:, k, 128 * m : 128 * m + 128],
                        wv_s[:, k, :],
                        start=(k == 0),
                        stop=(k == nk - 1),
                    )
                src = ps[:, :].rearrange("p (h d) -> p h d", h=hl)
                if on_act:
                    nc.scalar.copy(v_s[:, m, :, 0:64], src)
                else:
                    nc.vector.tensor_copy(v_s[:, m, :, 0:64], src)

            def emit_c(m, ks, dst_d):
                """Half of phase C for t-tile m, contracting head-pairs `ks`."""
                ob = pco.tile([128, kd], F32, name=f"ob{cnt[0]}", tag="ob")
                cnt[0] += 1
                for c in range(kd // 512):
                    ps = psp.tile(
                        [128, 512], F32, name=f"pc{cnt[0]}", tag="proj", bufs=2
                    )
                    cnt[0] += 1
                    for j, k in enumerate(ks):
                        nc.tensor.matmul(
                            ps[:, :],
                            ot_s[:, k, 128 * m : 128 * m + 128],
                            wo_s[:, k, 512 * c : 512 * c + 512],
                            start=(j == 0),
                            stop=(j == len(ks) - 1),
                        )
                    nc.vector.tensor_copy(ob[:, 512 * c : 512 * c + 512], ps[:, :])
                nc.sync.dma_start(dst_d[128 * m : 128 * m + 128, :], ob[:, :])

            # --------------- prologue: first Q/K chunk only ----------------
            emit_qk(wq_s, qt_s, 0, 0, on_act=True)
            emit_qk(wk_s, kt_s, 0, 0, on_act=True)

            # --------------- fused attention + filler ----------------------
            def emit_pv(h, ki, qc, a, b, ex, exo, otp):
                """PV for one (ki, qc) unit; ex columns [exo, exo + b - a)."""
                nc.tensor.matmul(
                    otp[0:65, a - 512 * qc : b - 512 * qc],
                    v_s[:, ki, h, :],
                    ex[:, exo : exo + b - a],
                    start=(ki == 0),
                    stop=(ki == 4 * qc + 3),
                )
                if ki != 4 * qc + 3:
                    return False
                mh, ph = h // 2, 64 * (h % 2)
                den = pbm.tile([1, 512], F32, name=f"dn{h}_{qc}", tag="den")
                nc.vector.tensor_copy(den[:, :], otp[64:65, :])
                rec = pbm.tile([1, 512], F32, name=f"rc{h}_{qc}", tag="rec")
                nc.vector.reciprocal_approx_fast(rec[:, :], den[:, :])
                bc = pbm.tile([64, 512], F32, name=f"bc{h}_{qc}", tag="bc")
                nc.gpsimd.partition_broadcast(bc[:, :], rec[0:1, :])
                cols = slice(512 * qc, 512 * qc + 512)
                if ph == 0:
                    nc.vector.tensor_mul(ot_s[0:64, mh, cols], otp[0:64, :], bc[:, :])
                else:
                    sc = pbm.tile([64, 512], BF16, name=f"sc{h}_{qc}", tag="sc")
                    nc.vector.tensor_mul(sc[:, :], otp[0:64, :], bc[:, :])
                    nc.sync.dma_start(ot_s[64:128, mh, cols], sc[:, :])
                return True

            ngrp_head = sum((4 * qc + 4) // 2 for qc in range(nqc))  # 20

            for p in range(dt):
                front = []
                spread = []
                if p == 0:
                    # dependency-safe front-load order (2 items per group):
                    # QK chunk n must precede attention q-chunk n; V tiles
                    # stream ahead of their PV consumers.
                    front = [
                        lambda: emit_v(0),
                        lambda: emit_v(1),
                        lambda: emit_qk(wq_s, qt_s, 0, 1),
                        lambda: emit_qk(wk_s, kt_s, 0, 1),
                        lambda: emit_v(2),
                        lambda: emit_v(3),
                        lambda: emit_v(4),
                        lambda: emit_qk(wq_s, qt_s, 0, 2),
                        lambda: emit_qk(wk_s, kt_s, 0, 2),
                        lambda: emit_v(5),
                        lambda: emit_v(6),
                        lambda: emit_v(7),
                        lambda: emit_qk(wq_s, qt_s, 0, 3),
                        lambda: emit_qk(wk_s, kt_s, 0, 3),
                    ] + [(lambda m=m: emit_v(m)) for m in range(8, mt)]
                if p < dt - 1:
                    for n in range(4):
                        spread.append(lambda n=n, p=p: emit_qk(wq_s, qt_s, p + 1, n))
                        spread.append(lambda n=n, p=p: emit_qk(wk_s, kt_s, p + 1, n))
                if p == dt - 1:
                    spread += [
                        (lambda m=m: emit_c(m, (0, 1), out2_d)) for m in range(mt)
                    ]
                fr = [0]
                fi = [0]
                pui = [0]
                npace = ngrp_head if p == dt - 1 else 2 * ngrp_head

                def maybe_fill():
                    pui[0] += 1
                    took = 0
                    while fr[0] < len(front) and took < 2:
                        front[fr[0]]()
                        fr[0] += 1
                        took += 1
                    if took:
                        return
                    want = pui[0] * len(spread) // npace
                    while fi[0] < min(want, len(spread)):
                        spread[fi[0]]()
                        fi[0] += 1

                for h in (2 * p, 2 * p + 1):
                    mh, ph = h // 2, 64 * (h % 2)
                    if p == dt - 1 and h == 2 * p + 1:
                        while fi[0] < len(spread):
                            spread[fi[0]]()
                            fi[0] += 1
                        spread = []
                        fi[0] = 0
                        pui[0] = 0
                    # deferred-PV depth: deep for the very first head so the
                    # ST/exp stream never blocks on the late-arriving V tiles
                    pv_depth = 12 if (p == 0 and h == 0) else 2
                    pv_pending = []

                    def drain_pv(limit):
                        while len(pv_pending) > limit:
                            args = pv_pending.pop(0)
                            done = emit_pv(*args)
                            if done and p == dt - 1 and h == 2 * p + 1:
                                dqc = args[2]
                                spread.extend(
                                    (lambda m=m: emit_c(m, (2, 3), out_d))
                                    for m in range(4 * dqc, 4 * dqc + 4)
                                )
                    for qc in range(nqc):
                        # otp bufs=2: before taking qc's slot (= qc-2's), all
                        # deferred PVs/normalize touching qc-2 must be emitted
                        while pv_pending and pv_pending[0][2] <= qc - 2:
                            drain_pv(len(pv_pending) - 1)
                        otp = psp.tile(
                            [65, 512], F32, name=f"otp{h}_{qc}", tag="ot", bufs=2
                        )
                        for ki0 in range(0, 4 * qc + 4, 2):
                            st = psp.tile(
                                [128, 1024], F32, name=f"st{h}_{ki0}_{qc}",
                                tag="st", bufs=2,
                            )
                            ex = pbe.tile(
                                [128, 1024], BF16, name=f"ex{h}_{ki0}_{qc}",
                                tag="ex",
                            )
                            # place the two units contiguously (no unwritten
                            # psum gap for exp): unit 2 at w1 if both fit in
                            # bank 0, else at the bank-1 boundary
                            b = 512 * qc + 512
                            a1 = max(128 * ki0, 512 * qc)
                            a2 = max(128 * (ki0 + 1), 512 * qc)
                            w1, w2 = b - a1, b - a2
                            o2 = w1 if w1 + w2 <= 512 else 512
                            ws = [(ki0, a1, 0), (ki0 + 1, a2, o2)]
                            for ki, a, off in ws:
                                nc.tensor.matmul(
                                    st[:, off : off + b - a],
                                    kt_s[ph : ph + 64, mh, 128 * ki : 128 * ki + 128],
                                    qt_s[ph : ph + 64, mh, a:b],
                                    start=True,
                                    stop=True,
                                )
                            nc.scalar.activation(
                                ex[:, 0 : o2 + w2], st[:, 0 : o2 + w2],
                                EXP, scale=scale,
                            )
                            for ki, a, off in ws:
                                if a == 128 * ki:
                                    nc.vector.tensor_mul(
                                        ex[:, off : off + 128],
                                        ex[:, off : off + 128],
                                        mask_t[:, :],
                                    )
                            maybe_fill()
                            pv_pending.extend(
                                (h, ki, qc, a, b, ex, off, otp)
                                for ki, a, off in ws
                            )
                            drain_pv(pv_depth)
                    drain_pv(0)
                while fi[0] < len(spread):
                    spread[fi[0]]()
                    fi[0] += 1

    nc.finalize()
    return nc


_NC_CACHE = {}


def _get_nc(key=(T, 512, 8, KD)):
    if key not in _NC_CACHE:
        _NC_CACHE[key] = build_nc(*key)
    return _NC_CACHE[key]


def make_in_maps(x, Wq, Wk, Wv, Wo, dl=512):
    in_maps = []
    for c in range(NCORES):
        b, g = c // 2, c % 2
        rows = slice(dl * g, dl * (g + 1))
        in_maps.append(
            {
                "xt": np.ascontiguousarray(x[b].T).astype(ml_dtypes.float8_e4m3fn),
                "xtb": np.ascontiguousarray(x[b].T).astype(ml_dtypes.bfloat16),
                "wq": np.ascontiguousarray(Wq[rows, :].T * WSCALE).astype(
                    ml_dtypes.float8_e4m3fn
                ),
                "wk": np.ascontiguousarray(Wk[rows, :].T * WSCALE).astype(
                    ml_dtypes.float8_e4m3fn
                ),
                "wv": np.ascontiguousarray(Wv[rows, :].T).astype(ml_dtypes.bfloat16),
                "wo": np.ascontiguousarray(Wo[:, rows].T).astype(ml_dtypes.bfloat16),
            }
        )
    return in_maps


def run_spmd(x, Wq, Wk, Wv, Wo, trace=False):
    nc = _get_nc()
    in_maps = make_in_maps(x, Wq, Wk, Wv, Wo)
    res = run_bass_kernel_spmd(nc, in_maps, list(range(NCORES)), trace=trace)
    outs = [
        res.results[c]["out"] + res.results[c]["out2"] for c in range(NCORES)
    ]
    final = np.stack([outs[2 * b] + outs[2 * b + 1] for b in range(B)])
    return final.astype(np.float32), res


def kernel(x, Wq, Wk, Wv, Wo):
    x = np.asarray(x, dtype=np.float32)
    Wq = np.asarray(Wq, dtype=np.float32)
    Wk = np.asarray(Wk, dtype=np.float32)
    Wv = np.asarray(Wv, dtype=np.float32)
    Wo = np.asarray(Wo, dtype=np.float32)
    out, _ = run_spmd(x, Wq, Wk, Wv, Wo)
    return out
